# revision 1
# baseline (speedup 1.0000x reference)
"""Multi-head self-attention on Trainium2, 8-way tensor-parallel over heads.

Problem: x[4,2048,1024], per-head Wq/Wk/Wv[16,1024,64] (+zero biases),
out = concat_h softmax(q_h k_h^T / 8) v_h  -> [4,2048,1024].

Sharding: head tensor-parallelism. Core c owns heads 2c and 2c+1; it gets the
full activations (pre-transposed to x^T on the host, cast to bf16) and its two
heads' weights (packed side by side), computes the full attention for those
heads, and writes two [4,2048,64] outputs. The host concatenates the 16 head
outputs along the feature dim.

Device dataflow per batch b (both heads packed where possible):
  QT/KT = (Wq|Wk packed [128,128] per D-chunk)^T @ xT   -> [128(2*dh), 2048] bf16
  V     = xT-chunk^T @ (Wv packed)                      -> [t,128] natural layout
  S^T   = K QT (row-tiled: head0 in PE rows 0-63, head1 in 64-127)
  E     = exp(S^T/8) on ScalarE (no max subtraction: scores are O(5))
  O'    = E^T-chunks @ [V|1]  (ones column makes col 64 the softmax denom)
  out   = O'[:, :64] * (1/O'[:, 64])   (per-partition scale, natural layout)
"""

import functools

import numpy as np
import ml_dtypes

import concourse.bass as bass  # noqa: F401  (AP types come through bacc)
import concourse.tile as tile
from concourse import bacc, mybir

B, S, D, H = 4, 2048, 1024, 16
DH = D // H  # 64
N_CORES = 8
HPC = H // N_CORES  # heads per core = 2
NCH = 8  # D chunks of 128
NT = S // 128  # 16 t-chunks
NBLK = 4  # s_i blocks of 512
BLK = S // NBLK

BF16 = mybir.dt.bfloat16
F32 = mybir.dt.float32
EXPFN = mybir.ActivationFunctionType.Exp
SCALE = 1.0 / np.sqrt(DH)


def _emit(ctx, tc, xt, wq, wk, wv, bqk, o0, o1):
    nc = tc.nc

    const = ctx.enter_context(tc.tile_pool(name="const", bufs=1))
    xt_pool = ctx.enter_context(tc.tile_pool(name="xt", bufs=2))
    qk_pool = ctx.enter_context(tc.tile_pool(name="qk", bufs=2))
    vh_pool = ctx.enter_context(tc.tile_pool(name="vh", bufs=2))
    e_pool = ctx.enter_context(tc.tile_pool(name="e", bufs=2))
    out_pool = ctx.enter_context(tc.tile_pool(name="out", bufs=4))
    mis_pool = ctx.enter_context(tc.tile_pool(name="mis", bufs=4))
    ps_s = ctx.enter_context(tc.tile_pool(name="ps_s", bufs=1, space="PSUM"))
    ps_o = ctx.enter_context(tc.tile_pool(name="ps_o", bufs=2, space="PSUM"))
    ps_p = ctx.enter_context(tc.tile_pool(name="ps_p", bufs=2, space="PSUM"))

    # Constants: packed weights [128, chunk, 128] and bias columns [128, 2]
    w_q = const.tile([128, NCH, 128], BF16, tag="wq")
    w_k = const.tile([128, NCH, 128], BF16, tag="wk")
    w_v = const.tile([128, NCH, 128], BF16, tag="wv")
    nc.sync.dma_start(w_q, wq.rearrange("c p m -> p c m"))
    nc.sync.dma_start(w_k, wk.rearrange("c p m -> p c m"))
    nc.sync.dma_start(w_v, wv.rearrange("c p m -> p c m"))
    bias = const.tile([128, 2], F32, tag="bias")
    nc.sync.dma_start(bias, bqk)

    for b in range(B):
        XT = xt_pool.tile([128, NCH, S], BF16, tag="xt")
        nc.sync.dma_start(XT, xt[b].rearrange("(c p) s -> p c s", p=128))

        # --- QT/KT projections: [128(=2 heads * 64), 2048] bf16 ---
        QT = qk_pool.tile([128, S], BF16, tag="qt")
        KT = qk_pool.tile([128, S], BF16, tag="kt")
        for dst, w_t, bcol in ((QT, w_q, bias[:, 0:1]), (KT, w_k, bias[:, 1:2])):
            for n in range(4):
                ps = ps_p.tile([128, 512], F32, tag="proj")
                for c in range(NCH):
                    nc.tensor.matmul(
                        ps,
                        w_t[:, c, :],
                        XT[:, c, n * 512 : (n + 1) * 512],
                        start=(c == 0),
                        stop=(c == NCH - 1),
                    )
                nc.vector.tensor_scalar_add(dst[:, n * 512 : (n + 1) * 512], ps, bcol)

        # --- V in natural layout [t, dh], one [128,16,66] tile per head;
        # col 64 = 1.0 (softmax denominator trick), col 65 pad ---
        V0 = vh_pool.tile([128, NT, 66], BF16, tag="v0")
        V1 = vh_pool.tile([128, NT, 66], BF16, tag="v1")
        nc.vector.memset(V0[:, :, 64:66], 1.0)
        nc.vector.memset(V1[:, :, 64:66], 1.0)
        for t in range(NT):
            ps = ps_p.tile([128, 128], F32, tag="proj")
            for c in range(NCH):
                nc.tensor.matmul(
                    ps,
                    XT[:, c, t * 128 : (t + 1) * 128],
                    w_v[:, c, :],
                    start=(c == 0),
                    stop=(c == NCH - 1),
                )
            nc.vector.tensor_copy(V0[:, t, 0:64], ps[:, 0:64])
            nc.vector.tensor_copy(V1[:, t, 0:64], ps[:, 64:128])

        # --- attention, pipelined over s_i blocks: scores/exp for block k are
        # interleaved with the AV matmuls of block k-1 so PE and ACT overlap ---
        E_prev = None
        for blk in range(NBLK + 1):
            units = []  # AV work units of the previous block
            if blk > 0:
                E0p, E1p = E_prev
                pblk = blk - 1
                for h, (Eh, Vh, oap) in enumerate(
                    ((E0p, V0, o0), (E1p, V1, o1))
                ):
                    for sc in range(4):
                        units.append((Eh, Vh, oap, pblk, sc))

            E_cur = None
            if blk < NBLK:
                E0 = e_pool.tile([128, NT, BLK], BF16, tag="e0")
                E1 = e_pool.tile([128, NT, BLK], BF16, tag="e1")
                E_cur = (E0, E1)
                q0 = QT[0:64, blk * BLK : (blk + 1) * BLK]
                q1 = QT[64:128, blk * BLK : (blk + 1) * BLK]

            for g in range(8):
                if blk < NBLK:
                    psa = ps_s.tile([128, 2, BLK], F32, tag="s0")
                    psb = ps_s.tile([128, 2, BLK], F32, tag="s1")
                    for j in range(2):
                        t = 2 * g + j
                        nc.tensor.matmul(
                            psa[:, j],
                            KT[0:64, t * 128 : (t + 1) * 128],
                            q0,
                            start=True,
                            stop=True,
                        )
                        nc.tensor.matmul(
                            psb[:, j],
                            KT[64:128, t * 128 : (t + 1) * 128],
                            q1,
                            start=True,
                            stop=True,
                        )
                    nc.scalar.activation(E0[:, 2 * g : 2 * g + 2, :], psa, EXPFN, scale=SCALE)
                    nc.scalar.activation(E1[:, 2 * g : 2 * g + 2, :], psb, EXPFN, scale=SCALE)

                if g < len(units):
                    Eh, Vh, oap, pblk, sc = units[g]
                    ps = ps_o.tile([128, 65], F32, tag="o")
                    for t in range(NT):
                        nc.tensor.matmul(
                            ps,
                            Eh[:, t, sc * 128 : (sc + 1) * 128],
                            Vh[:, t, 0:65],
                            start=(t == 0),
                            stop=(t == NT - 1),
                        )
                    rcp = mis_pool.tile([128, 1], F32, tag="rcp")
                    nc.vector.reciprocal(rcp, ps[:, 64:65])
                    ot = out_pool.tile([128, 64], F32, tag="ot")
                    nc.vector.tensor_scalar_mul(ot, ps[:, 0:64], rcp)
                    r0 = pblk * BLK + sc * 128
                    nc.sync.dma_start(oap[b, r0 : r0 + 128, :], ot)

            E_prev = E_cur


def _build():
    from contextlib import ExitStack

    nc = bacc.Bacc("TRN2", target_bir_lowering=False, debug=False, num_devices=N_CORES)
    xt = nc.dram_tensor("xt", [B, D, S], BF16, kind="ExternalInput").ap()
    wq = nc.dram_tensor("wq", [NCH, 128, 128], BF16, kind="ExternalInput").ap()
    wk = nc.dram_tensor("wk", [NCH, 128, 128], BF16, kind="ExternalInput").ap()
    wv = nc.dram_tensor("wv", [NCH, 128, 128], BF16, kind="ExternalInput").ap()
    bqk = nc.dram_tensor("bqk", [128, 2], F32, kind="ExternalInput").ap()
    o0 = nc.dram_tensor("o0", [B, S, DH], F32, kind="ExternalOutput").ap()
    o1 = nc.dram_tensor("o1", [B, S, DH], F32, kind="ExternalOutput").ap()

    with tile.TileContext(nc) as tc:
        with ExitStack() as ctx:
            _emit(ctx, tc, xt, wq, wk, wv, bqk, o0, o1)
    nc.compile()
    return nc


def _pack_inputs(x, Wq, bq, Wk, bk, Wv, bv):
    """Host-side shard/layout prep. Returns per-core input maps."""
    bf16 = ml_dtypes.bfloat16
    x = np.asarray(x, dtype=np.float32)
    xt = np.ascontiguousarray(x.transpose(0, 2, 1)).astype(bf16)  # [B, D, S]

    def packw(W, c):
        # [D, 2*DH] -> [NCH, 128, 128], heads 2c, 2c+1 side by side
        Wp = np.concatenate([W[2 * c], W[2 * c + 1]], axis=1)
        return np.ascontiguousarray(Wp.reshape(NCH, 128, 128)).astype(bf16)

    in_maps = []
    for c in range(N_CORES):
        bias = np.stack(
            [
                np.concatenate([bq[2 * c], bq[2 * c + 1]]),
                np.concatenate([bk[2 * c], bk[2 * c + 1]]),
            ],
            axis=1,
        ).astype(np.float32)  # [128, 2]
        in_maps.append(
            {
                "xt": xt,
                "wq": packw(np.asarray(Wq, np.float32), c),
                "wk": packw(np.asarray(Wk, np.float32), c),
                "wv": packw(np.asarray(Wv, np.float32), c),
                "bqk": bias,
            }
        )
    return in_maps


@functools.lru_cache(maxsize=1)
def _runner():
    """Compile the bass program once and build a cached jitted SPMD callable."""
    import jax
    from jax.experimental.shard_map import shard_map
    from jax.sharding import Mesh, PartitionSpec

    from concourse.bass2jax import (
        _bass_exec_p,
        install_neuronx_cc_hook,
        partition_id_tensor,
    )

    nc = _build()
    install_neuronx_cc_hook()

    partition_name = nc.partition_id_tensor.name if nc.partition_id_tensor else None
    in_names, out_names, out_avals = [], [], []
    for alloc in nc.m.functions[0].allocations:
        if not isinstance(alloc, mybir.MemoryLocationSet):
            continue
        name = alloc.memorylocations[0].name
        if alloc.kind == "ExternalInput":
            if name != partition_name:
                in_names.append(name)
        elif alloc.kind == "ExternalOutput":
            out_names.append(name)
            out_avals.append(
                jax.core.ShapedArray(
                    tuple(alloc.tensor_shape), mybir.dt.np(alloc.dtype)
                )
            )
    n_params = len(in_names)
    all_names = in_names + out_names
    if partition_name is not None:
        all_names = all_names + [partition_name]

    def _body(*args):
        operands = list(args)
        if partition_name is not None:
            operands.append(partition_id_tensor())
        outs = _bass_exec_p.bind(
            *operands,
            out_avals=tuple(out_avals),
            in_names=tuple(all_names),
            out_names=tuple(out_names),
            lowering_input_output_aliases=(),
            sim_require_finite=True,
            sim_require_nnan=True,
            nc=nc,
        )
        return tuple(outs)

    devices = jax.devices()[:N_CORES]
    mesh = Mesh(np.asarray(devices), ("core",))
    n_outs = len(out_names)
    sharded = jax.jit(
        shard_map(
            _body,
            mesh=mesh,
            in_specs=(PartitionSpec("core"),) * (n_params + n_outs),
            out_specs=(PartitionSpec("core"),) * n_outs,
            check_rep=False,
        ),
        keep_unused=True,
    )
    return nc, sharded, in_names, out_names, out_avals


def _stage(in_maps):
    """Concatenate per-core inputs along axis 0 (shard_map convention)."""
    _, _, in_names, out_names, out_avals = _runner()
    concat_in = [
        np.concatenate([in_maps[c][name] for c in range(N_CORES)], axis=0)
        for name in in_names
    ]
    concat_zeros = [
        np.zeros((N_CORES * a.shape[0], *a.shape[1:]), a.dtype) for a in out_avals
    ]
    return concat_in, concat_zeros


def _execute(concat_in, concat_zeros):
    _, sharded, _, out_names, out_avals = _runner()
    out_arrs = sharded(*concat_in, *concat_zeros)
    per_core = [
        {
            name: np.asarray(out_arrs[i]).reshape(N_CORES, *out_avals[i].shape)[c]
            for i, name in enumerate(out_names)
        }
        for c in range(N_CORES)
    ]
    return per_core


def kernel(x, Wq, bq, Wk, bk, Wv, bv):
    x = np.asarray(x, np.float32)
    Wq, bq = np.asarray(Wq, np.float32), np.asarray(bq, np.float32)
    Wk, bk = np.asarray(Wk, np.float32), np.asarray(bk, np.float32)
    Wv, bv = np.asarray(Wv, np.float32), np.asarray(bv, np.float32)

    in_maps = _pack_inputs(x, Wq, bq, Wk, bk, Wv, bv)
    results = _execute(*_stage(in_maps))

    out = np.empty((B, S, D), np.float32)
    for c in range(N_CORES):
        out[..., (2 * c) * DH : (2 * c + 1) * DH] = results[c]["o0"]
        out[..., (2 * c + 1) * DH : (2 * c + 2) * DH] = results[c]["o1"]
    if np.any(bv):
        # rows of softmax sum to 1, so the v-bias adds directly to the output
        out += bv.reshape(H * DH)[None, None, :]
    return out


# revision 13
# speedup vs baseline: 18.2777x; 18.2777x over previous
"""Multi-head self-attention on Trainium2, 8-way tensor-parallel over heads.

Problem: x[4,2048,1024], per-head Wq/Wk/Wv[16,1024,64] (+zero biases),
out = concat_h softmax(q_h k_h^T / 8) v_h  -> [4,2048,1024].

Sharding: head tensor-parallelism. Core c owns heads 2c and 2c+1; it gets the
full activations (pre-transposed to x^T on the host, cast to bf16) and its two
heads' weights (packed side by side), computes the full attention for those
heads, and writes two [4,2048,64] outputs. The host concatenates the 16 head
outputs along the feature dim.

Device dataflow per batch b (both heads packed where possible):
  QT/KT = (Wq|Wk packed [128,128] per D-chunk)^T @ xT   -> [128(2*dh), 2048] bf16
  V     = xT-chunk^T @ (Wv packed)                      -> [t,128] natural layout
  S^T   = K QT (row-tiled: head0 in PE rows 0-63, head1 in 64-127)
  E     = exp(S^T/8) on ScalarE (no max subtraction: scores are O(5))
  O'    = E^T-chunks @ [V|1]  (ones column makes col 64 the softmax denom)
  out   = O'[:, :64] * (1/O'[:, 64])   (per-partition scale, natural layout)
"""

import functools

import numpy as np
import ml_dtypes

import concourse.bass as bass  # noqa: F401  (AP types come through bacc)
import concourse.tile as tile
from concourse import bacc, mybir

B, S, D, H = 4, 2048, 1024, 16
DH = D // H  # 64
N_CORES = 8
HPC = H // N_CORES  # heads per core = 2
NCH = 8  # D chunks of 128
NT = S // 128  # 16 t-chunks
NBLK = 4  # s_i blocks of 512
BLK = S // NBLK

BF16 = mybir.dt.bfloat16
F32 = mybir.dt.float32
EXPFN = mybir.ActivationFunctionType.Exp
SCALE = 1.0 / np.sqrt(DH)

# debug knobs for perturbation profiling (leave all False for production)
SKIP_AV = False
SKIP_SCORES = False
SKIP_EXP = False
SKIP_EPI = False


def _emit(ctx, tc, xt, wq, wk, wv, bqk, o0, o1, repeat=1):
    nc = tc.nc

    const = ctx.enter_context(tc.tile_pool(name="const", bufs=1))
    xt_pool = ctx.enter_context(tc.tile_pool(name="xt", bufs=2))
    qk_pool = ctx.enter_context(tc.tile_pool(name="qk", bufs=2))
    vh_pool = ctx.enter_context(tc.tile_pool(name="vh", bufs=2))
    e_pool = ctx.enter_context(tc.tile_pool(name="e", bufs=2))
    out_pool = ctx.enter_context(tc.tile_pool(name="out", bufs=4))
    mis_pool = ctx.enter_context(tc.tile_pool(name="mis", bufs=4))
    ps_s = ctx.enter_context(tc.tile_pool(name="ps_s", bufs=1, space="PSUM"))
    ps_o = ctx.enter_context(tc.tile_pool(name="ps_o", bufs=2, space="PSUM"))
    ps_p = ctx.enter_context(tc.tile_pool(name="ps_p", bufs=2, space="PSUM"))

    # Constants: packed weights [128, chunk, 128] and bias columns [128, 2].
    # Loaded via the gpsimd DMA path so they don't delay the batch-0 x
    # prefetch on the sync HWDGE queue.
    w_q = const.tile([128, NCH, 128], BF16, tag="wq")
    w_k = const.tile([128, NCH, 128], BF16, tag="wk")
    w_v = const.tile([128, NCH, 128], BF16, tag="wv")
    nc.gpsimd.dma_start(w_k, wk.rearrange("c p m -> p c m"))
    nc.gpsimd.dma_start(w_q, wq.rearrange("c p m -> p c m"))
    nc.gpsimd.dma_start(w_v, wv.rearrange("c p m -> p c m"))
    bias = const.tile([128, 2], F32, tag="bias")
    nc.gpsimd.dma_start(bias, bqk)

    def make_proj_units(b):
        """Allocate batch-b tiles and return (tiles, unit closures).

        The XT prefetch DMA is emitted immediately; the compute units are
        invoked one per attention slot of the previous batch so the
        projection matmuls fill PE gaps while ScalarE drains exps.
        """
        XT = xt_pool.tile([128, NCH, S], BF16, tag="xt")
        for c in range(NCH):
            nc.sync.dma_start(XT[:, c, :], xt[b, c * 128 : (c + 1) * 128, :])
        QT = qk_pool.tile([128, S], BF16, tag="qt")
        KT = qk_pool.tile([128, S], BF16, tag="kt")
        V0 = vh_pool.tile([128, NT, 66], BF16, tag="v0")
        V1 = vh_pool.tile([128, NT, 66], BF16, tag="v1")

        units = []

        def u_qk(dst, w_t, bcol, n):
            ps = ps_p.tile([128, 512], F32, tag="proj")
            for c in range(NCH):
                nc.tensor.matmul(
                    ps,
                    w_t[:, c, :],
                    XT[:, c, n * 512 : (n + 1) * 512],
                    start=(c == 0),
                    stop=(c == NCH - 1),
                )
            nc.vector.tensor_scalar_add(dst[:, n * 512 : (n + 1) * 512], ps, bcol)

        for dst, w_t, bcol in ((QT, w_q, bias[:, 0:1]), (KT, w_k, bias[:, 1:2])):
            for n in range(4):
                units.append(functools.partial(u_qk, dst, w_t, bcol, n))

        def u_ones():
            nc.vector.memset(V0[:, :, 64:66], 1.0)
            nc.vector.memset(V1[:, :, 64:66], 1.0)

        units.append(u_ones)

        def u_v(t):
            ps = ps_p.tile([128, 128], F32, tag="proj")
            for c in range(NCH):
                nc.tensor.matmul(
                    ps,
                    XT[:, c, t * 128 : (t + 1) * 128],
                    w_v[:, c, :],
                    start=(c == 0),
                    stop=(c == NCH - 1),
                )
            nc.vector.tensor_copy(V0[:, t, 0:64], ps[:, 0:64])
            nc.vector.tensor_copy(V1[:, t, 0:64], ps[:, 64:128])

        for t in range(NT):
            units.append(functools.partial(u_v, t))
        return (QT, KT, V0, V1), units

    batches = [b for _ in range(repeat) for b in range(B)]
    tiles, units0 = make_proj_units(batches[0])
    # Minimal prologue: K n-tile 0 and Q n-tile 0 are all the first scores
    # group needs. The rest is paced into batch 0's early attention slots
    # (3/slot) so ScalarE starts exping ~25us earlier.
    units0[4]()  # K proj n=0
    units0[0]()  # Q proj n=0
    own0 = [units0[i] for i in (5, 1, 6, 2, 7, 3)] + units0[8:]

    for bi, b in enumerate(batches):
        QT, KT, V0, V1 = tiles
        if bi + 1 < len(batches):
            tiles_next, pending = make_proj_units(batches[bi + 1])
        else:
            tiles_next, pending = None, []
        nslots = (NBLK + 1) * 8
        queues = [(pending, nslots - len(pending), 1)]
        if bi == 0:
            queues.insert(0, (own0, 0, 3))
        slot = 0

        # --- attention, pipelined over s_i blocks: scores/exp for block k are
        # interleaved with the AV matmuls of block k-1 so PE and ACT overlap ---
        E_prev = None
        for blk in range(NBLK + 1):
            units = []  # AV work units of the previous block
            if blk > 0:
                E0p, E1p = E_prev
                pblk = blk - 1
                for h, (Eh, Vh, oap) in enumerate(
                    ((E0p, V0, o0), (E1p, V1, o1))
                ):
                    for sc in range(4):
                        units.append((Eh, Vh, oap, pblk, sc))

            E_cur = None
            if blk < NBLK:
                E0 = e_pool.tile([128, NT, BLK], BF16, tag="e0")
                E1 = e_pool.tile([128, NT, BLK], BF16, tag="e1")
                E_cur = (E0, E1)
                q0 = QT[0:64, blk * BLK : (blk + 1) * BLK]
                q1 = QT[64:128, blk * BLK : (blk + 1) * BLK]

            for g in range(8):
                if blk < NBLK and not SKIP_SCORES:
                    psa = ps_s.tile([128, 2, BLK], F32, tag="s0")
                    psb = ps_s.tile([128, 2, BLK], F32, tag="s1")
                    for j in range(2):
                        t = 2 * g + j
                        nc.tensor.matmul(
                            psa[:, j],
                            KT[0:64, t * 128 : (t + 1) * 128],
                            q0,
                            start=True,
                            stop=True,
                        )
                        nc.tensor.matmul(
                            psb[:, j],
                            KT[64:128, t * 128 : (t + 1) * 128],
                            q1,
                            start=True,
                            stop=True,
                        )
                    if not SKIP_EXP:
                        nc.scalar.activation(E0[:, 2 * g : 2 * g + 2, :], psa, EXPFN, scale=SCALE)
                        nc.scalar.activation(E1[:, 2 * g : 2 * g + 2, :], psb, EXPFN, scale=SCALE)

                if g < len(units) and not SKIP_AV:
                    Eh, Vh, oap, pblk, sc = units[g]
                    ps = ps_o.tile([128, 65], F32, tag="o")
                    for t in range(NT):
                        nc.tensor.matmul(
                            ps,
                            Eh[:, t, sc * 128 : (sc + 1) * 128],
                            Vh[:, t, 0:65],
                            start=(t == 0),
                            stop=(t == NT - 1),
                        )
                    if not SKIP_EPI:
                        rcp = mis_pool.tile([128, 1], F32, tag="rcp")
                        nc.vector.reciprocal(rcp, ps[:, 64:65])
                        ot = out_pool.tile([128, 64], F32, tag="ot")
                        nc.vector.tensor_scalar_mul(ot, ps[:, 0:64], rcp)
                        r0 = pblk * BLK + sc * 128
                        nc.sync.dma_start(oap[b, r0 : r0 + 128, :], ot)

                for q, start_slot, pace in queues:
                    if slot >= start_slot:
                        for _ in range(min(pace, len(q))):
                            q.pop(0)()
                slot += 1

            E_prev = E_cur
        tiles = tiles_next


def _build(repeat=1):
    from contextlib import ExitStack

    nc = bacc.Bacc("TRN2", target_bir_lowering=False, debug=False, num_devices=N_CORES)
    xt = nc.dram_tensor("xt", [B, D, S], BF16, kind="ExternalInput").ap()
    wq = nc.dram_tensor("wq", [NCH, 128, 128], BF16, kind="ExternalInput").ap()
    wk = nc.dram_tensor("wk", [NCH, 128, 128], BF16, kind="ExternalInput").ap()
    wv = nc.dram_tensor("wv", [NCH, 128, 128], BF16, kind="ExternalInput").ap()
    bqk = nc.dram_tensor("bqk", [128, 2], F32, kind="ExternalInput").ap()
    o0 = nc.dram_tensor("o0", [B, S, DH], F32, kind="ExternalOutput").ap()
    o1 = nc.dram_tensor("o1", [B, S, DH], F32, kind="ExternalOutput").ap()

    with tile.TileContext(nc) as tc:
        with ExitStack() as ctx:
            _emit(ctx, tc, xt, wq, wk, wv, bqk, o0, o1, repeat=repeat)
    nc.compile()
    return nc


def _pack_inputs(x, Wq, bq, Wk, bk, Wv, bv):
    """Host-side shard/layout prep. Returns per-core input maps."""
    bf16 = ml_dtypes.bfloat16
    x = np.asarray(x, dtype=np.float32)
    xt = np.ascontiguousarray(x.transpose(0, 2, 1)).astype(bf16)  # [B, D, S]

    def packw(W, c):
        # [D, 2*DH] -> [NCH, 128, 128], heads 2c, 2c+1 side by side
        Wp = np.concatenate([W[2 * c], W[2 * c + 1]], axis=1)
        return np.ascontiguousarray(Wp.reshape(NCH, 128, 128)).astype(bf16)

    in_maps = []
    for c in range(N_CORES):
        bias = np.stack(
            [
                np.concatenate([bq[2 * c], bq[2 * c + 1]]),
                np.concatenate([bk[2 * c], bk[2 * c + 1]]),
            ],
            axis=1,
        ).astype(np.float32)  # [128, 2]
        in_maps.append(
            {
                "xt": xt,
                "wq": packw(np.asarray(Wq, np.float32), c),
                "wk": packw(np.asarray(Wk, np.float32), c),
                "wv": packw(np.asarray(Wv, np.float32), c),
                "bqk": bias,
            }
        )
    return in_maps


@functools.lru_cache(maxsize=1)
def _runner():
    """Compile the bass program once and build a cached jitted SPMD callable."""
    import jax
    from jax.experimental.shard_map import shard_map
    from jax.sharding import Mesh, PartitionSpec

    from concourse.bass2jax import (
        _bass_exec_p,
        install_neuronx_cc_hook,
        partition_id_tensor,
    )

    nc = _build()
    install_neuronx_cc_hook()

    partition_name = nc.partition_id_tensor.name if nc.partition_id_tensor else None
    in_names, out_names, out_avals = [], [], []
    for alloc in nc.m.functions[0].allocations:
        if not isinstance(alloc, mybir.MemoryLocationSet):
            continue
        name = alloc.memorylocations[0].name
        if alloc.kind == "ExternalInput":
            if name != partition_name:
                in_names.append(name)
        elif alloc.kind == "ExternalOutput":
            out_names.append(name)
            out_avals.append(
                jax.core.ShapedArray(
                    tuple(alloc.tensor_shape), mybir.dt.np(alloc.dtype)
                )
            )
    n_params = len(in_names)
    all_names = in_names + out_names
    if partition_name is not None:
        all_names = all_names + [partition_name]

    def _body(*args):
        operands = list(args)
        if partition_name is not None:
            operands.append(partition_id_tensor())
        outs = _bass_exec_p.bind(
            *operands,
            out_avals=tuple(out_avals),
            in_names=tuple(all_names),
            out_names=tuple(out_names),
            lowering_input_output_aliases=(),
            sim_require_finite=True,
            sim_require_nnan=True,
            nc=nc,
        )
        return tuple(outs)

    devices = jax.devices()[:N_CORES]
    mesh = Mesh(np.asarray(devices), ("core",))
    n_outs = len(out_names)
    sharded = jax.jit(
        shard_map(
            _body,
            mesh=mesh,
            in_specs=(PartitionSpec("core"),) * (n_params + n_outs),
            out_specs=(PartitionSpec("core"),) * n_outs,
            check_rep=False,
        ),
        keep_unused=True,
    )
    return nc, sharded, in_names, out_names, out_avals


def _stage(in_maps):
    """Concatenate per-core inputs along axis 0 (shard_map convention)."""
    _, _, in_names, out_names, out_avals = _runner()
    concat_in = [
        np.concatenate([in_maps[c][name] for c in range(N_CORES)], axis=0)
        for name in in_names
    ]
    concat_zeros = [
        np.zeros((N_CORES * a.shape[0], *a.shape[1:]), a.dtype) for a in out_avals
    ]
    return concat_in, concat_zeros


def _execute(concat_in, concat_zeros):
    _, sharded, _, out_names, out_avals = _runner()
    out_arrs = sharded(*concat_in, *concat_zeros)
    per_core = [
        {
            name: np.asarray(out_arrs[i]).reshape(N_CORES, *out_avals[i].shape)[c]
            for i, name in enumerate(out_names)
        }
        for c in range(N_CORES)
    ]
    return per_core


def kernel(x, Wq, bq, Wk, bk, Wv, bv):
    x = np.asarray(x, np.float32)
    Wq, bq = np.asarray(Wq, np.float32), np.asarray(bq, np.float32)
    Wk, bk = np.asarray(Wk, np.float32), np.asarray(bk, np.float32)
    Wv, bv = np.asarray(Wv, np.float32), np.asarray(bv, np.float32)

    in_maps = _pack_inputs(x, Wq, bq, Wk, bk, Wv, bv)
    results = _execute(*_stage(in_maps))

    out = np.empty((B, S, D), np.float32)
    for c in range(N_CORES):
        out[..., (2 * c) * DH : (2 * c + 1) * DH] = results[c]["o0"]
        out[..., (2 * c + 1) * DH : (2 * c + 2) * DH] = results[c]["o1"]
    if np.any(bv):
        # rows of softmax sum to 1, so the v-bias adds directly to the output
        out += bv.reshape(H * DH)[None, None, :]
    return out


# revision 15
# speedup vs baseline: 19.7492x; 1.0805x over previous
"""Multi-head self-attention on Trainium2, 8-way tensor-parallel over heads.

Problem: x[4,2048,1024], per-head Wq/Wk/Wv[16,1024,64] (+zero biases),
out = concat_h softmax(q_h k_h^T / 8) v_h  -> [4,2048,1024].

Sharding: head tensor-parallelism. Core c owns heads 2c and 2c+1; it gets the
full activations (pre-transposed to x^T on the host, cast to bf16) and its two
heads' weights (packed side by side), computes the full attention for those
heads, and writes two [4,2048,64] outputs. The host concatenates the 16 head
outputs along the feature dim.

Device dataflow per batch b (both heads packed where possible):
  QT/KT = (Wq|Wk packed [128,128] per D-chunk)^T @ xT   -> [128(2*dh), 2048] bf16
  V     = xT-chunk^T @ (Wv packed)                      -> [t,128] natural layout
  S^T   = K QT (row-tiled: head0 in PE rows 0-63, head1 in 64-127)
  E     = exp(S^T/8) on ScalarE (no max subtraction: scores are O(5))
  O'    = E^T-chunks @ [V|1]  (ones column makes col 64 the softmax denom)
  out   = O'[:, :64] * (1/O'[:, 64])   (per-partition scale, natural layout)
"""

import functools

import numpy as np
import ml_dtypes

import concourse.bass as bass  # noqa: F401  (AP types come through bacc)
import concourse.tile as tile
from concourse import bacc, mybir

B, S, D, H = 4, 2048, 1024, 16
DH = D // H  # 64
N_CORES = 8
HPC = H // N_CORES  # heads per core = 2
NCH = 8  # D chunks of 128
NT = S // 128  # 16 t-chunks
NBLK = 4  # s_i blocks of 512
BLK = S // NBLK

BF16 = mybir.dt.bfloat16
F32 = mybir.dt.float32
EXPFN = mybir.ActivationFunctionType.Exp
SCALE = 1.0 / np.sqrt(DH)

# debug knobs for perturbation profiling (leave all False for production)
SKIP_AV = False
SKIP_SCORES = False
SKIP_EXP = False
SKIP_EPI = False


def _emit(ctx, tc, xt, wq, wk, wv, bqk, o0, o1, repeat=1):
    nc = tc.nc

    const = ctx.enter_context(tc.tile_pool(name="const", bufs=1))
    xt_pool = ctx.enter_context(tc.tile_pool(name="xt", bufs=2))
    qk_pool = ctx.enter_context(tc.tile_pool(name="qk", bufs=2))
    vh_pool = ctx.enter_context(tc.tile_pool(name="vh", bufs=2))
    e_pool = ctx.enter_context(tc.tile_pool(name="e", bufs=2))
    out_pool = ctx.enter_context(tc.tile_pool(name="out", bufs=4))
    mis_pool = ctx.enter_context(tc.tile_pool(name="mis", bufs=4))
    ps_s = ctx.enter_context(tc.tile_pool(name="ps_s", bufs=1, space="PSUM"))
    ps_o = ctx.enter_context(tc.tile_pool(name="ps_o", bufs=2, space="PSUM"))
    ps_p = ctx.enter_context(tc.tile_pool(name="ps_p", bufs=2, space="PSUM"))

    # Constants: packed weights [128, chunk, 128] and bias columns [128, 2].
    # Loaded via the gpsimd DMA path so they don't delay the batch-0 x
    # prefetch on the sync HWDGE queue.
    w_q = const.tile([128, NCH, 128], BF16, tag="wq")
    w_k = const.tile([128, NCH, 128], BF16, tag="wk")
    w_v = const.tile([128, NCH, 128], BF16, tag="wv")
    nc.gpsimd.dma_start(w_k, wk.rearrange("c p m -> p c m"))
    nc.gpsimd.dma_start(w_q, wq.rearrange("c p m -> p c m"))
    nc.gpsimd.dma_start(w_v, wv.rearrange("c p m -> p c m"))
    bias = const.tile([128, 2], F32, tag="bias")
    nc.gpsimd.dma_start(bias, bqk)

    def make_proj_units(b):
        """Allocate batch-b tiles and return (tiles, unit closures).

        The XT prefetch DMA is emitted immediately; the compute units are
        invoked one per attention slot of the previous batch so the
        projection matmuls fill PE gaps while ScalarE drains exps.
        """
        XT = xt_pool.tile([128, NCH, S], BF16, tag="xt")
        for c in range(NCH):
            nc.sync.dma_start(XT[:, c, :], xt[b, c * 128 : (c + 1) * 128, :])
        QT = qk_pool.tile([128, S], BF16, tag="qt")
        KT = qk_pool.tile([128, S], BF16, tag="kt")
        V0 = vh_pool.tile([128, NT, 66], BF16, tag="v0")
        V1 = vh_pool.tile([128, NT, 66], BF16, tag="v1")

        units = []

        def u_qk(dst, w_t, bcol, n):
            ps = ps_p.tile([128, 512], F32, tag="proj")
            for c in range(NCH):
                nc.tensor.matmul(
                    ps,
                    w_t[:, c, :],
                    XT[:, c, n * 512 : (n + 1) * 512],
                    start=(c == 0),
                    stop=(c == NCH - 1),
                )
            nc.vector.tensor_scalar_add(dst[:, n * 512 : (n + 1) * 512], ps, bcol)

        for dst, w_t, bcol in ((QT, w_q, bias[:, 0:1]), (KT, w_k, bias[:, 1:2])):
            for n in range(4):
                units.append(functools.partial(u_qk, dst, w_t, bcol, n))

        def u_ones():
            nc.vector.memset(V0[:, :, 64:66], 1.0)
            nc.vector.memset(V1[:, :, 64:66], 1.0)

        units.append(u_ones)

        def u_v(t):
            ps = ps_p.tile([128, 128], F32, tag="proj")
            for c in range(NCH):
                nc.tensor.matmul(
                    ps,
                    XT[:, c, t * 128 : (t + 1) * 128],
                    w_v[:, c, :],
                    start=(c == 0),
                    stop=(c == NCH - 1),
                )
            nc.vector.tensor_copy(V0[:, t, 0:64], ps[:, 0:64])
            nc.vector.tensor_copy(V1[:, t, 0:64], ps[:, 64:128])

        for t in range(NT):
            units.append(functools.partial(u_v, t))
        return (QT, KT, V0, V1), units

    batches = [b for _ in range(repeat) for b in range(B)]
    tiles, units0 = make_proj_units(batches[0])
    # Minimal prologue: K n-tile 0 and Q n-tile 0 are all the first scores
    # group needs. The rest is paced into batch 0's early attention slots
    # (3/slot) so ScalarE starts exping ~25us earlier.
    units0[4]()  # K proj n=0
    units0[0]()  # Q proj n=0
    own0 = [units0[i] for i in (5, 1, 6, 2, 7, 3)] + units0[8:]

    for bi, b in enumerate(batches):
        QT, KT, V0, V1 = tiles
        if bi + 1 < len(batches):
            tiles_next, pending = make_proj_units(batches[bi + 1])
        else:
            tiles_next, pending = None, []
        nslots = (NBLK + 1) * 8
        queues = [(pending, nslots - len(pending), lambda s: 1)]
        if bi == 0:
            # 2/slot while the remaining QK projections drain (keeps ScalarE
            # fed with early scores), then 4/slot so V is ready by slot 8
            queues.insert(0, (own0, 0, lambda s: 2 if s < 3 else 4))
        slot = 0

        # --- attention, pipelined over s_i blocks: scores/exp for block k are
        # interleaved with the AV matmuls of block k-1 so PE and ACT overlap ---
        E_prev = None
        for blk in range(NBLK + 1):
            units = []  # AV work units of the previous block
            if blk > 0:
                E0p, E1p = E_prev
                pblk = blk - 1
                for h, (Eh, Vh, oap) in enumerate(
                    ((E0p, V0, o0), (E1p, V1, o1))
                ):
                    for sc in range(4):
                        units.append((Eh, Vh, oap, pblk, sc))

            E_cur = None
            if blk < NBLK:
                E0 = e_pool.tile([128, NT, BLK], BF16, tag="e0")
                E1 = e_pool.tile([128, NT, BLK], BF16, tag="e1")
                E_cur = (E0, E1)
                q0 = QT[0:64, blk * BLK : (blk + 1) * BLK]
                q1 = QT[64:128, blk * BLK : (blk + 1) * BLK]

            for g in range(8):
                if blk < NBLK and not SKIP_SCORES:
                    psa = ps_s.tile([128, 2, BLK], F32, tag="s0")
                    psb = ps_s.tile([128, 2, BLK], F32, tag="s1")
                    for j in range(2):
                        t = 2 * g + j
                        nc.tensor.matmul(
                            psa[:, j],
                            KT[0:64, t * 128 : (t + 1) * 128],
                            q0,
                            start=True,
                            stop=True,
                        )
                        nc.tensor.matmul(
                            psb[:, j],
                            KT[64:128, t * 128 : (t + 1) * 128],
                            q1,
                            start=True,
                            stop=True,
                        )
                    if not SKIP_EXP:
                        nc.scalar.activation(E0[:, 2 * g : 2 * g + 2, :], psa, EXPFN, scale=SCALE)
                        nc.scalar.activation(E1[:, 2 * g : 2 * g + 2, :], psb, EXPFN, scale=SCALE)

                if g < len(units) and not SKIP_AV:
                    Eh, Vh, oap, pblk, sc = units[g]
                    ps = ps_o.tile([128, 65], F32, tag="o")
                    for t in range(NT):
                        nc.tensor.matmul(
                            ps,
                            Eh[:, t, sc * 128 : (sc + 1) * 128],
                            Vh[:, t, 0:65],
                            start=(t == 0),
                            stop=(t == NT - 1),
                        )
                    if not SKIP_EPI:
                        rcp = mis_pool.tile([128, 1], F32, tag="rcp")
                        nc.vector.reciprocal(rcp, ps[:, 64:65])
                        ot = out_pool.tile([128, 64], F32, tag="ot")
                        nc.vector.tensor_scalar_mul(ot, ps[:, 0:64], rcp)
                        r0 = pblk * BLK + sc * 128
                        nc.sync.dma_start(oap[b, r0 : r0 + 128, :], ot)

                for q, start_slot, pace in queues:
                    if slot >= start_slot:
                        for _ in range(min(pace(slot), len(q))):
                            q.pop(0)()
                slot += 1

            E_prev = E_cur
        tiles = tiles_next


def _build(repeat=1):
    from contextlib import ExitStack

    nc = bacc.Bacc("TRN2", target_bir_lowering=False, debug=False, num_devices=N_CORES)
    xt = nc.dram_tensor("xt", [B, D, S], BF16, kind="ExternalInput").ap()
    wq = nc.dram_tensor("wq", [NCH, 128, 128], BF16, kind="ExternalInput").ap()
    wk = nc.dram_tensor("wk", [NCH, 128, 128], BF16, kind="ExternalInput").ap()
    wv = nc.dram_tensor("wv", [NCH, 128, 128], BF16, kind="ExternalInput").ap()
    bqk = nc.dram_tensor("bqk", [128, 2], F32, kind="ExternalInput").ap()
    o0 = nc.dram_tensor("o0", [B, S, DH], F32, kind="ExternalOutput").ap()
    o1 = nc.dram_tensor("o1", [B, S, DH], F32, kind="ExternalOutput").ap()

    with tile.TileContext(nc) as tc:
        with ExitStack() as ctx:
            _emit(ctx, tc, xt, wq, wk, wv, bqk, o0, o1, repeat=repeat)
    nc.compile()
    return nc


def _pack_inputs(x, Wq, bq, Wk, bk, Wv, bv):
    """Host-side shard/layout prep. Returns per-core input maps."""
    bf16 = ml_dtypes.bfloat16
    x = np.asarray(x, dtype=np.float32)
    xt = np.ascontiguousarray(x.transpose(0, 2, 1)).astype(bf16)  # [B, D, S]

    def packw(W, c):
        # [D, 2*DH] -> [NCH, 128, 128], heads 2c, 2c+1 side by side
        Wp = np.concatenate([W[2 * c], W[2 * c + 1]], axis=1)
        return np.ascontiguousarray(Wp.reshape(NCH, 128, 128)).astype(bf16)

    in_maps = []
    for c in range(N_CORES):
        bias = np.stack(
            [
                np.concatenate([bq[2 * c], bq[2 * c + 1]]),
                np.concatenate([bk[2 * c], bk[2 * c + 1]]),
            ],
            axis=1,
        ).astype(np.float32)  # [128, 2]
        in_maps.append(
            {
                "xt": xt,
                "wq": packw(np.asarray(Wq, np.float32), c),
                "wk": packw(np.asarray(Wk, np.float32), c),
                "wv": packw(np.asarray(Wv, np.float32), c),
                "bqk": bias,
            }
        )
    return in_maps


@functools.lru_cache(maxsize=1)
def _runner():
    """Compile the bass program once and build a cached jitted SPMD callable."""
    import jax
    from jax.experimental.shard_map import shard_map
    from jax.sharding import Mesh, PartitionSpec

    from concourse.bass2jax import (
        _bass_exec_p,
        install_neuronx_cc_hook,
        partition_id_tensor,
    )

    nc = _build()
    install_neuronx_cc_hook()

    partition_name = nc.partition_id_tensor.name if nc.partition_id_tensor else None
    in_names, out_names, out_avals = [], [], []
    for alloc in nc.m.functions[0].allocations:
        if not isinstance(alloc, mybir.MemoryLocationSet):
            continue
        name = alloc.memorylocations[0].name
        if alloc.kind == "ExternalInput":
            if name != partition_name:
                in_names.append(name)
        elif alloc.kind == "ExternalOutput":
            out_names.append(name)
            out_avals.append(
                jax.core.ShapedArray(
                    tuple(alloc.tensor_shape), mybir.dt.np(alloc.dtype)
                )
            )
    n_params = len(in_names)
    all_names = in_names + out_names
    if partition_name is not None:
        all_names = all_names + [partition_name]

    def _body(*args):
        operands = list(args)
        if partition_name is not None:
            operands.append(partition_id_tensor())
        outs = _bass_exec_p.bind(
            *operands,
            out_avals=tuple(out_avals),
            in_names=tuple(all_names),
            out_names=tuple(out_names),
            lowering_input_output_aliases=(),
            sim_require_finite=True,
            sim_require_nnan=True,
            nc=nc,
        )
        return tuple(outs)

    devices = jax.devices()[:N_CORES]
    mesh = Mesh(np.asarray(devices), ("core",))
    n_outs = len(out_names)
    sharded = jax.jit(
        shard_map(
            _body,
            mesh=mesh,
            in_specs=(PartitionSpec("core"),) * (n_params + n_outs),
            out_specs=(PartitionSpec("core"),) * n_outs,
            check_rep=False,
        ),
        keep_unused=True,
    )
    return nc, sharded, in_names, out_names, out_avals


def _stage(in_maps):
    """Concatenate per-core inputs along axis 0 (shard_map convention)."""
    _, _, in_names, out_names, out_avals = _runner()
    concat_in = [
        np.concatenate([in_maps[c][name] for c in range(N_CORES)], axis=0)
        for name in in_names
    ]
    concat_zeros = [
        np.zeros((N_CORES * a.shape[0], *a.shape[1:]), a.dtype) for a in out_avals
    ]
    return concat_in, concat_zeros


def _execute(concat_in, concat_zeros):
    _, sharded, _, out_names, out_avals = _runner()
    out_arrs = sharded(*concat_in, *concat_zeros)
    per_core = [
        {
            name: np.asarray(out_arrs[i]).reshape(N_CORES, *out_avals[i].shape)[c]
            for i, name in enumerate(out_names)
        }
        for c in range(N_CORES)
    ]
    return per_core


def kernel(x, Wq, bq, Wk, bk, Wv, bv):
    x = np.asarray(x, np.float32)
    Wq, bq = np.asarray(Wq, np.float32), np.asarray(bq, np.float32)
    Wk, bk = np.asarray(Wk, np.float32), np.asarray(bk, np.float32)
    Wv, bv = np.asarray(Wv, np.float32), np.asarray(bv, np.float32)

    in_maps = _pack_inputs(x, Wq, bq, Wk, bk, Wv, bv)
    results = _execute(*_stage(in_maps))

    out = np.empty((B, S, D), np.float32)
    for c in range(N_CORES):
        out[..., (2 * c) * DH : (2 * c + 1) * DH] = results[c]["o0"]
        out[..., (2 * c + 1) * DH : (2 * c + 2) * DH] = results[c]["o1"]
    if np.any(bv):
        # rows of softmax sum to 1, so the v-bias adds directly to the output
        out += bv.reshape(H * DH)[None, None, :]
    return out
